# revision 8
# baseline (speedup 1.0000x reference)
"""MultiHeadClassifier (MoE routing) Trainium2 kernel.

Problem: B=65536 samples of dim D=1024, each routed by task_id to one of
T=16 two-layer heads (D->H=128 relu -> C=10). The dense reference computes
all 16 heads for every sample (275 GFLOP); here we route on the host and
compute only each sample's own head (~17 GFLOP), data-parallel with 2 tasks
per NeuronCore across 8 cores.

Per-core budget (measured): x-stream DMA ~17.3MB bf16 at ~320-360 GB/s is
the roofline (~50us); PE time (bf16 L1 8 matmuls/512-subtile + L2) is just
under it. Design goals, from baseline trace analysis:
  - One flat DRAM buffer per core laid out in exact consumption order;
    x arrives in ~12 block DMAs (0.25-2MB) on the sync HWDGE ring instead
    of 48 chunk DMAs -> kills per-DMA SDMA-engine boundary bubbles.
  - Tapered block sizes (512 head, 1024 middle, 512/256/128 tail): PE can
    start early and the post-stream tail is tiny.
  - Weights packed to 2 DMAs/slot on the scalar HWDGE ring (lands first);
    outputs per block on gpsimd SWDGE (own queue rows, never blocks x).
  - Short PE warmup on the weight tile itself (no memset), timed to end
    as the first x block lands, so HAM is at K=8/8 for all real matmuls.
  - Fewer tiles/DMAs/instructions -> fewer Tile semaphores -> shorter
    kernel-tail drain+barrier butterfly (it waits per allocated sem).
"""

import sys

import numpy as np

for _p in ("/opt/trn_rl_repo", "/root/.axon_site/_ro/trn_rl_repo"):
    if _p not in sys.path:
        sys.path.append(_p)

import concourse.bacc as bacc
import concourse.mybir as mybir
from concourse.bass_utils import run_bass_kernel_spmd
from concourse.tile import TileContext

B, D, T, H, C = 65536, 1024, 16, 128, 10
N_CORES = 8
S = T // N_CORES  # task slots per core = 2
DC = D // 128  # d-chunks of 128 = 8
MT = 512  # m-subtile (max fp32 PSUM free dim)
WCOLS = DC * H + 16  # w1 (1024) + w2 (10) + pad

MM_DTYPE = "bf16"

_F32 = mybir.dt.float32
_BF16 = mybir.dt.bfloat16


def _np_bf16():
    import ml_dtypes

    return np.dtype(ml_dtypes.bfloat16)


def _plan_blocks(M):
    """Tapered block sizes summing to M: small head (PE starts early),
    1024-col middle, descending tail (short post-stream critical path)."""
    assert M % 32 == 0
    if M <= 1024:
        return [M]
    head, tail = [256, 512], [512, 256, 128]
    rem = M - sum(head) - sum(tail)
    if rem < 0:
        head, tail = [], [512, 256, 128]
        rem = M - sum(tail)
        if rem < 0:
            return [M - 256, 256] if M > 512 else [M]
    mids = []
    while rem > 0:
        c = min(1024, rem)
        if rem - c and rem - c < 256:
            c = rem - 256
        mids.append(c)
        rem -= c
    return head + mids + tail


def _chunks(total, step):
    out = []
    p = 0
    while p < total:
        c = min(step, total - p)
        out.append((p, c))
        p += c
    return out


def _build(M_task, mm_dtype=MM_DTYPE):
    assert mm_dtype == "bf16"
    blocks = _plan_blocks(M_task)
    # interleaved issue order: (b0,s0), (b0,s1), (b1,s0), ...
    sched = [(bi, s) for bi in range(len(blocks)) for s in range(S)]
    # flat x layout: per (block, slot) region [128, DC*xl], partition-major
    offs = {}
    off = 0
    starts = np.concatenate([[0], np.cumsum(blocks)]).astype(int)
    for bi, s in sched:
        xl = blocks[bi]
        offs[(bi, s)] = off
        off += 128 * DC * xl
    total_x = off

    nc = bacc.Bacc(None, target_bir_lowering=False)
    xL = nc.declare_dram_parameter("xL", [total_x], _BF16, isOutput=False)
    wcat = nc.declare_dram_parameter("wcat", [S, 128, WCOLS], _BF16, isOutput=False)
    bcat = nc.declare_dram_parameter("bcat", [128, S * 128], _F32, isOutput=False)
    outT = nc.declare_dram_parameter("outT", [S, C, M_task], _F32, isOutput=True)

    relu = mybir.ActivationFunctionType.Relu
    N_WARMUP = 5

    with TileContext(nc) as tc:
        with (
            tc.tile_pool(name="wpool", bufs=1) as wpool,
            tc.tile_pool(name="xpool", bufs=1) as xpool,
            tc.tile_pool(name="hpool", bufs=8) as hpool,
            tc.tile_pool(name="opool", bufs=1) as opool,
            tc.tile_pool(name="psum1", bufs=5, space="PSUM") as psum1,
            tc.tile_pool(name="psum2", bufs=2, space="PSUM") as psum2,
            tc.tile_pool(name="psumw", bufs=1, space="PSUM") as psumw,
        ):
            # weights + biases on the scalar HWDGE ring (slot-0 weights
            # first: they gate the PE warmup at ~9us); the sync ring carries
            # ONLY the x stream so its first block lands asap. Biases ship as
            # one [128, 256] f32 DMA (values replicated to 512B rows: tiny
            # 8B-row DMAs are descriptor-RMW-slow and would delay wcat1)
            w0t = wpool.tile([128, WCOLS], _BF16, tag="w0", name="wt0")
            nc.scalar.dma_start(w0t, wcat[0])
            bt = wpool.tile([128, S * 128], _F32, tag="bb", name="bt")
            nc.scalar.dma_start(bt, bcat[:, :])
            w1t = wpool.tile([128, WCOLS], _BF16, tag="w1", name="wt1")
            nc.scalar.dma_start(w1t, wcat[1])
            wts = [
                (w0t, bt[:, 0:1], bt[0:C, 64:65]),
                (w1t, bt[:, 128:129], bt[0:C, 192:193]),
            ]

            # all x block DMAs up-front on the sync HWDGE ring, in consumption
            # order: the ring FIFO delivers blocks sequentially at line rate
            xts = {}
            for bi, s in sched:
                xl = blocks[bi]
                o = offs[(bi, s)]
                xt = xpool.tile(
                    [128, DC * xl], _BF16, tag=f"x{bi}_{s}", name=f"x{bi}_{s}"
                )
                nc.sync.dma_start(
                    xt, xL[o : o + 128 * DC * xl].rearrange("(p k) -> p k", p=128)
                )
                xts[(bi, s)] = xt

            # PE warmup on slot-0 weights (lands ~8.2us; first x block ~12us):
            # garbage matmuls into a scratch bank release the HAM clock gate
            wps = psumw.tile([128, MT], _F32, tag="wps")
            w0 = wts[0][0]
            for _ in range(N_WARMUP):
                nc.tensor.matmul(wps[:], w0[:, :128], w0[:, :MT], start=True, stop=True)

            for bi, s in sched:
                xl = blocks[bi]
                x0 = starts[bi]
                wt, b1t, b2t = wts[s]
                xt = xts[(bi, s)]
                ot = opool.tile([C, xl], _F32, tag=f"o{bi}_{s}", name=f"o{bi}_{s}")
                subs = _chunks(xl, MT)
                # waves of <=4 m-subtiles; dc-outer within a wave so the
                # stationary W1 chunk is reused across the wave's matmuls
                for w0i in range(0, len(subs), 4):
                    wave = subs[w0i : w0i + 4]
                    ps1s = [
                        psum1.tile([H, MT], _F32, tag="ps1", name=f"ps1_{j}")
                        for j in range(len(wave))
                    ]
                    for dc in range(DC):
                        lhs = wt[:, dc * H : (dc + 1) * H]
                        for j, (m0, mt) in enumerate(wave):
                            nc.tensor.matmul(
                                ps1s[j][:, :mt],
                                lhs,
                                xt[:, dc * xl + m0 : dc * xl + m0 + mt],
                                start=(dc == 0),
                                stop=(dc == DC - 1),
                            )
                    # relu+b1 for the whole wave first, then the wave's L2
                    # matmuls back-to-back into disjoint 32-col strips of the
                    # PE array (col tiling): they run concurrently, ~1 matmul
                    # cost for up to 4
                    hts = []
                    for j, (m0, mt) in enumerate(wave):
                        ht = hpool.tile([H, MT], _BF16, tag="h")
                        nc.scalar.activation(
                            ht[:, :mt], ps1s[j][:, :mt], relu, bias=b1t
                        )
                        hts.append(ht)
                    ps2 = psum2.tile([128, MT], _F32, tag="ps2")
                    for j, (m0, mt) in enumerate(wave):
                        nc.tensor.matmul(
                            ps2[32 * j : 32 * j + C, :mt],
                            wt[:, DC * H : DC * H + C],
                            hts[j][:, :mt],
                            start=True,
                            stop=True,
                            tile_position=(0, 32 * j),
                        )
                    for j, (m0, mt) in enumerate(wave):
                        nc.vector.tensor_tensor(
                            ot[:, m0 : m0 + mt],
                            ps2[32 * j : 32 * j + C, :mt],
                            b2t.to_broadcast([C, mt]),
                            mybir.AluOpType.add,
                        )
                # sync ring: out descriptors queue behind the x stream and
                # drain right after it; ot tiles are never recycled (distinct
                # tags) so compute never waits on these
                nc.sync.dma_start(outT[s, :, x0 : x0 + xl], ot[:])
    nc.compile()
    return nc


def _prepare(x, task_id, W1, b1, W2, b2, mm_dtype=MM_DTYPE):
    """Host-side routing: returns (in_maps, idx, counts, M_task)."""
    bf16 = _np_bf16()
    x = np.ascontiguousarray(np.asarray(x, dtype=np.float32))
    task_id = np.asarray(task_id).astype(np.int64)
    W1 = np.asarray(W1, dtype=np.float32)
    b1 = np.asarray(b1, dtype=np.float32)
    W2 = np.asarray(W2, dtype=np.float32)
    b2 = np.asarray(b2, dtype=np.float32)

    order = np.argsort(task_id, kind="stable")
    counts = np.bincount(task_id, minlength=T)
    starts_t = np.concatenate([[0], np.cumsum(counts)])
    M_task = max(128, int(-(-int(counts.max()) // 32) * 32))

    blocks = _plan_blocks(M_task)
    sched = [(bi, s) for bi in range(len(blocks)) for s in range(S)]
    bstarts = np.concatenate([[0], np.cumsum(blocks)]).astype(int)

    # idx[t] = sample rows for task t, padded with row 0 (discarded later)
    idx = np.zeros((T, M_task), dtype=np.int64)
    for t in range(T):
        idx[t, : counts[t]] = order[starts_t[t] : starts_t[t + 1]]

    in_maps = []
    for c in range(N_CORES):
        ts_c = [S * c + s for s in range(S)]
        # xT[s] = [DC, 128, M] (d-major within chunk on axis 1)
        xTs = []
        for s in range(S):
            xg = x[idx[ts_c[s]]].astype(bf16)  # [M, D]
            xTs.append(np.ascontiguousarray(xg.T).reshape(DC, 128, M_task))
        xL = np.empty(sum(128 * DC * b for b in blocks) * S, dtype=bf16)
        off = 0
        for bi, s in sched:
            xl = blocks[bi]
            x0 = bstarts[bi]
            # region [128, DC, xl] partition-major
            reg = xTs[s][:, :, x0 : x0 + xl].transpose(1, 0, 2)
            n = 128 * DC * xl
            xL[off : off + n] = reg.reshape(-1)
            off += n

        wcat = np.zeros((S, 128, WCOLS), dtype=bf16)
        bcat = np.zeros((128, S * 128), dtype=np.float32)
        for s in range(S):
            t = ts_c[s]
            # w1 [D,H] -> [128, DC*H] partition-major
            wcat[s, :, : DC * H] = (
                W1[t].reshape(DC, 128, H).transpose(1, 0, 2).reshape(128, DC * H)
            ).astype(bf16)
            wcat[s, :, DC * H : DC * H + C] = W2[t].astype(bf16)
            bcat[:, s * 128 : s * 128 + 64] = b1[t][:, None]
            bcat[:C, s * 128 + 64 : s * 128 + 128] = b2[t][:, None]

        in_maps.append({"xL": xL, "wcat": wcat, "bcat": bcat})
    return in_maps, idx, counts, M_task


def _unshard(results, idx, counts, b_total=B):
    out = np.empty((b_total, C), dtype=np.float32)
    for c in range(N_CORES):
        yT = np.asarray(results[c]["outT"])  # [S, C, M_task]
        y = yT.transpose(0, 2, 1)  # [S, M_task, C]
        for s in range(S):
            t = S * c + s
            cnt = counts[t]
            out[idx[t, :cnt]] = y[s, :cnt]
    return out


def kernel(x, task_id, W1, b1, W2, b2):
    in_maps, idx, counts, M_task = _prepare(x, task_id, W1, b1, W2, b2)
    nc = _build(M_task)
    try:
        res = run_bass_kernel_spmd(nc, in_maps, list(range(N_CORES)))
    except Exception:
        # transient NRT device hiccups (e.g. NRT_EXEC_UNIT_UNRECOVERABLE)
        # have been observed to succeed on retry
        res = run_bass_kernel_spmd(nc, in_maps, list(range(N_CORES)))
    return _unshard(res.results, idx, counts, b_total=np.asarray(task_id).shape[0])


# revision 9
# speedup vs baseline: 1.1069x; 1.1069x over previous
"""MultiHeadClassifier (MoE routing) Trainium2 kernel.

Problem: B=65536 samples of dim D=1024, each routed by task_id to one of
T=16 two-layer heads (D->H=128 relu -> C=10). The dense reference computes
all 16 heads for every sample (275 GFLOP); here we route on the host and
compute only each sample's own head (~17 GFLOP), data-parallel with 2 tasks
per NeuronCore across 8 cores.

Per-core budget (measured): x-stream DMA ~17.3MB bf16 at ~320-360 GB/s is
the roofline (~50us); PE time (bf16 L1 8 matmuls/512-subtile + L2) is just
under it. Design goals, from baseline trace analysis:
  - One flat DRAM buffer per core laid out in exact consumption order;
    x arrives in ~12 block DMAs (0.25-2MB) on the sync HWDGE ring instead
    of 48 chunk DMAs -> kills per-DMA SDMA-engine boundary bubbles.
  - Tapered block sizes (512 head, 1024 middle, 512/256/128 tail): PE can
    start early and the post-stream tail is tiny.
  - Weights packed to 2 DMAs/slot on the scalar HWDGE ring (lands first);
    outputs per block on gpsimd SWDGE (own queue rows, never blocks x).
  - Short PE warmup on the weight tile itself (no memset), timed to end
    as the first x block lands, so HAM is at K=8/8 for all real matmuls.
  - Fewer tiles/DMAs/instructions -> fewer Tile semaphores -> shorter
    kernel-tail drain+barrier butterfly (it waits per allocated sem).
"""

import sys

import numpy as np

for _p in ("/opt/trn_rl_repo", "/root/.axon_site/_ro/trn_rl_repo"):
    if _p not in sys.path:
        sys.path.append(_p)

import concourse.bacc as bacc
import concourse.mybir as mybir
from concourse.bass_utils import run_bass_kernel_spmd
from concourse.tile import TileContext

B, D, T, H, C = 65536, 1024, 16, 128, 10
N_CORES = 8
S = T // N_CORES  # task slots per core = 2
DC = D // 128  # d-chunks of 128 = 8
MT = 512  # m-subtile (max fp32 PSUM free dim)
WCOLS = DC * H + 16  # w1 (1024) + w2 (10) + pad

MM_DTYPE = "bf16"

_F32 = mybir.dt.float32
_BF16 = mybir.dt.bfloat16


def _np_bf16():
    import ml_dtypes

    return np.dtype(ml_dtypes.bfloat16)


def _plan_blocks(M):
    """Tapered block sizes summing to M: small head (PE starts early),
    1024-col middle, descending tail (short post-stream critical path)."""
    assert M % 32 == 0
    if M <= 1024:
        return [M]
    head, tail = [256, 512], [512, 256, 128]
    rem = M - sum(head) - sum(tail)
    if rem < 0:
        head, tail = [], [512, 256, 128]
        rem = M - sum(tail)
        if rem < 0:
            return [M - 256, 256] if M > 512 else [M]
    mids = []
    while rem > 0:
        c = min(1024, rem)
        if rem - c and rem - c < 256:
            c = rem - 256
        mids.append(c)
        rem -= c
    return head + mids + tail


def _chunks(total, step):
    out = []
    p = 0
    while p < total:
        c = min(step, total - p)
        out.append((p, c))
        p += c
    return out


def _build(M_task, mm_dtype=MM_DTYPE):
    assert mm_dtype == "bf16"
    blocks = _plan_blocks(M_task)
    # interleaved issue order: (b0,s0), (b0,s1), (b1,s0), ...
    sched = [(bi, s) for bi in range(len(blocks)) for s in range(S)]
    # flat x layout: per (block, slot) region [128, DC*xl], partition-major
    offs = {}
    off = 0
    starts = np.concatenate([[0], np.cumsum(blocks)]).astype(int)
    for bi, s in sched:
        xl = blocks[bi]
        offs[(bi, s)] = off
        off += 128 * DC * xl
    total_x = off

    nc = bacc.Bacc(None, target_bir_lowering=False)
    xL = nc.declare_dram_parameter("xL", [total_x], _BF16, isOutput=False)
    wcat = nc.declare_dram_parameter("wcat", [S, 128, WCOLS], _BF16, isOutput=False)
    bcat = nc.declare_dram_parameter("bcat", [128, S * 128], _F32, isOutput=False)
    outT = nc.declare_dram_parameter("outT", [S, C, M_task], _F32, isOutput=True)

    relu = mybir.ActivationFunctionType.Relu
    N_WARMUP = 5

    with TileContext(nc) as tc:
        with (
            tc.tile_pool(name="wpool", bufs=1) as wpool,
            tc.tile_pool(name="xpool", bufs=1) as xpool,
            tc.tile_pool(name="hpool", bufs=8) as hpool,
            tc.tile_pool(name="opool", bufs=1) as opool,
            tc.tile_pool(name="psum1", bufs=5, space="PSUM") as psum1,
            tc.tile_pool(name="psum2", bufs=2, space="PSUM") as psum2,
            tc.tile_pool(name="psumw", bufs=1, space="PSUM") as psumw,
        ):
            # weights + biases FIRST on the sync ring, ahead of the x
            # flood (~1.6us of stream): anywhere else they drain packet-
            # interleaved with x and land 10+us late, stalling slot-1.
            # Biases ship as one [128, 256] f32 DMA (values replicated to
            # 512B rows: 8B-row DMAs are descriptor-RMW-slow)
            w0t = wpool.tile([128, WCOLS], _BF16, tag="w0", name="wt0")
            nc.sync.dma_start(w0t, wcat[0])
            w1t = wpool.tile([128, WCOLS], _BF16, tag="w1", name="wt1")
            nc.sync.dma_start(w1t, wcat[1])
            bt = wpool.tile([128, S * 128], _F32, tag="bb", name="bt")
            nc.sync.dma_start(bt, bcat[:, :])
            wts = [
                (w0t, bt[:, 0:1], bt[0:C, 64:65]),
                (w1t, bt[:, 128:129], bt[0:C, 192:193]),
            ]

            # all x block DMAs up-front on the sync HWDGE ring, in consumption
            # order: the ring FIFO delivers blocks sequentially at line rate
            xts = {}
            for bi, s in sched:
                xl = blocks[bi]
                o = offs[(bi, s)]
                xt = xpool.tile(
                    [128, DC * xl], _BF16, tag=f"x{bi}_{s}", name=f"x{bi}_{s}"
                )
                nc.sync.dma_start(
                    xt, xL[o : o + 128 * DC * xl].rearrange("(p k) -> p k", p=128)
                )
                xts[(bi, s)] = xt

            # PE warmup on slot-0 weights (lands ~8.2us; first x block ~12us):
            # garbage matmuls into a scratch bank release the HAM clock gate
            wps = psumw.tile([128, MT], _F32, tag="wps")
            w0 = wts[0][0]
            for _ in range(N_WARMUP):
                nc.tensor.matmul(wps[:], w0[:, :128], w0[:, :MT], start=True, stop=True)

            for bi, s in sched:
                xl = blocks[bi]
                x0 = starts[bi]
                wt, b1t, b2t = wts[s]
                xt = xts[(bi, s)]
                ot = opool.tile([C, xl], _F32, tag=f"o{bi}_{s}", name=f"o{bi}_{s}")
                subs = _chunks(xl, MT)
                # waves of <=4 m-subtiles; dc-outer within a wave so the
                # stationary W1 chunk is reused across the wave's matmuls
                for w0i in range(0, len(subs), 4):
                    wave = subs[w0i : w0i + 4]
                    ps1s = [
                        psum1.tile([H, MT], _F32, tag="ps1", name=f"ps1_{j}")
                        for j in range(len(wave))
                    ]
                    for dc in range(DC):
                        lhs = wt[:, dc * H : (dc + 1) * H]
                        for j, (m0, mt) in enumerate(wave):
                            nc.tensor.matmul(
                                ps1s[j][:, :mt],
                                lhs,
                                xt[:, dc * xl + m0 : dc * xl + m0 + mt],
                                start=(dc == 0),
                                stop=(dc == DC - 1),
                            )
                    # relu+b1 for the whole wave first, then the wave's L2
                    # matmuls back-to-back into disjoint 32-col strips of the
                    # PE array (col tiling): they run concurrently, ~1 matmul
                    # cost for up to 4
                    hts = []
                    for j, (m0, mt) in enumerate(wave):
                        ht = hpool.tile([H, MT], _BF16, tag="h")
                        nc.scalar.activation(
                            ht[:, :mt], ps1s[j][:, :mt], relu, bias=b1t
                        )
                        hts.append(ht)
                    ps2 = psum2.tile([128, MT], _F32, tag="ps2")
                    for j, (m0, mt) in enumerate(wave):
                        nc.tensor.matmul(
                            ps2[32 * j : 32 * j + C, :mt],
                            wt[:, DC * H : DC * H + C],
                            hts[j][:, :mt],
                            start=True,
                            stop=True,
                            tile_position=(0, 32 * j),
                        )
                    for j, (m0, mt) in enumerate(wave):
                        nc.vector.tensor_tensor(
                            ot[:, m0 : m0 + mt],
                            ps2[32 * j : 32 * j + C, :mt],
                            b2t.to_broadcast([C, mt]),
                            mybir.AluOpType.add,
                        )
                # sync ring: out descriptors queue behind the x stream and
                # drain right after it; ot tiles are never recycled (distinct
                # tags) so compute never waits on these
                nc.sync.dma_start(outT[s, :, x0 : x0 + xl], ot[:])
    nc.compile()
    return nc


def _prepare(x, task_id, W1, b1, W2, b2, mm_dtype=MM_DTYPE):
    """Host-side routing: returns (in_maps, idx, counts, M_task)."""
    bf16 = _np_bf16()
    x = np.ascontiguousarray(np.asarray(x, dtype=np.float32))
    task_id = np.asarray(task_id).astype(np.int64)
    W1 = np.asarray(W1, dtype=np.float32)
    b1 = np.asarray(b1, dtype=np.float32)
    W2 = np.asarray(W2, dtype=np.float32)
    b2 = np.asarray(b2, dtype=np.float32)

    order = np.argsort(task_id, kind="stable")
    counts = np.bincount(task_id, minlength=T)
    starts_t = np.concatenate([[0], np.cumsum(counts)])
    M_task = max(128, int(-(-int(counts.max()) // 32) * 32))

    blocks = _plan_blocks(M_task)
    sched = [(bi, s) for bi in range(len(blocks)) for s in range(S)]
    bstarts = np.concatenate([[0], np.cumsum(blocks)]).astype(int)

    # idx[t] = sample rows for task t, padded with row 0 (discarded later)
    idx = np.zeros((T, M_task), dtype=np.int64)
    for t in range(T):
        idx[t, : counts[t]] = order[starts_t[t] : starts_t[t + 1]]

    in_maps = []
    for c in range(N_CORES):
        ts_c = [S * c + s for s in range(S)]
        # xT[s] = [DC, 128, M] (d-major within chunk on axis 1)
        xTs = []
        for s in range(S):
            xg = x[idx[ts_c[s]]].astype(bf16)  # [M, D]
            xTs.append(np.ascontiguousarray(xg.T).reshape(DC, 128, M_task))
        xL = np.empty(sum(128 * DC * b for b in blocks) * S, dtype=bf16)
        off = 0
        for bi, s in sched:
            xl = blocks[bi]
            x0 = bstarts[bi]
            # region [128, DC, xl] partition-major
            reg = xTs[s][:, :, x0 : x0 + xl].transpose(1, 0, 2)
            n = 128 * DC * xl
            xL[off : off + n] = reg.reshape(-1)
            off += n

        wcat = np.zeros((S, 128, WCOLS), dtype=bf16)
        bcat = np.zeros((128, S * 128), dtype=np.float32)
        for s in range(S):
            t = ts_c[s]
            # w1 [D,H] -> [128, DC*H] partition-major
            wcat[s, :, : DC * H] = (
                W1[t].reshape(DC, 128, H).transpose(1, 0, 2).reshape(128, DC * H)
            ).astype(bf16)
            wcat[s, :, DC * H : DC * H + C] = W2[t].astype(bf16)
            bcat[:, s * 128 : s * 128 + 64] = b1[t][:, None]
            bcat[:C, s * 128 + 64 : s * 128 + 128] = b2[t][:, None]

        in_maps.append({"xL": xL, "wcat": wcat, "bcat": bcat})
    return in_maps, idx, counts, M_task


def _unshard(results, idx, counts, b_total=B):
    out = np.empty((b_total, C), dtype=np.float32)
    for c in range(N_CORES):
        yT = np.asarray(results[c]["outT"])  # [S, C, M_task]
        y = yT.transpose(0, 2, 1)  # [S, M_task, C]
        for s in range(S):
            t = S * c + s
            cnt = counts[t]
            out[idx[t, :cnt]] = y[s, :cnt]
    return out


def kernel(x, task_id, W1, b1, W2, b2):
    in_maps, idx, counts, M_task = _prepare(x, task_id, W1, b1, W2, b2)
    nc = _build(M_task)
    try:
        res = run_bass_kernel_spmd(nc, in_maps, list(range(N_CORES)))
    except Exception:
        # transient NRT device hiccups (e.g. NRT_EXEC_UNIT_UNRECOVERABLE)
        # have been observed to succeed on retry
        res = run_bass_kernel_spmd(nc, in_maps, list(range(N_CORES)))
    return _unshard(res.results, idx, counts, b_total=np.asarray(task_id).shape[0])


# revision 11
# speedup vs baseline: 1.1103x; 1.0030x over previous
"""MultiHeadClassifier (MoE routing) Trainium2 kernel.

Problem: B=65536 samples of dim D=1024, each routed by task_id to one of
T=16 two-layer heads (D->H=128 relu -> C=10). The dense reference computes
all 16 heads for every sample (275 GFLOP); here we route on the host and
compute only each sample's own head (~17 GFLOP), data-parallel with 2 tasks
per NeuronCore across 8 cores.

Per-core budget (measured): x-stream DMA ~17.3MB bf16 at ~320-360 GB/s is
the roofline (~50us); PE time (bf16 L1 8 matmuls/512-subtile + L2) is just
under it. Design goals, from baseline trace analysis:
  - One flat DRAM buffer per core laid out in exact consumption order;
    x arrives in ~12 block DMAs (0.25-2MB) on the sync HWDGE ring instead
    of 48 chunk DMAs -> kills per-DMA SDMA-engine boundary bubbles.
  - Tapered block sizes (512 head, 1024 middle, 512/256/128 tail): PE can
    start early and the post-stream tail is tiny.
  - Weights packed to 2 DMAs/slot on the scalar HWDGE ring (lands first);
    outputs per block on gpsimd SWDGE (own queue rows, never blocks x).
  - Short PE warmup on the weight tile itself (no memset), timed to end
    as the first x block lands, so HAM is at K=8/8 for all real matmuls.
  - Fewer tiles/DMAs/instructions -> fewer Tile semaphores -> shorter
    kernel-tail drain+barrier butterfly (it waits per allocated sem).
"""

import sys

import numpy as np

for _p in ("/opt/trn_rl_repo", "/root/.axon_site/_ro/trn_rl_repo"):
    if _p not in sys.path:
        sys.path.append(_p)

import concourse.bacc as bacc
import concourse.mybir as mybir
from concourse.bass_utils import run_bass_kernel_spmd
from concourse.tile import TileContext

B, D, T, H, C = 65536, 1024, 16, 128, 10
N_CORES = 8
S = T // N_CORES  # task slots per core = 2
DC = D // 128  # d-chunks of 128 = 8
MT = 512  # m-subtile (max fp32 PSUM free dim)
WCOLS = DC * H + 16  # w1 (1024) + w2 (10) + pad

MM_DTYPE = "bf16"

_F32 = mybir.dt.float32
_BF16 = mybir.dt.bfloat16


def _np_bf16():
    import ml_dtypes

    return np.dtype(ml_dtypes.bfloat16)


def _plan_blocks(M):
    """Tapered block sizes summing to M: small head (PE starts early),
    1024-col middle, descending tail (short post-stream critical path)."""
    assert M % 32 == 0
    if M <= 1024:
        return [M]
    head, tail = [512, 512], [512, 256, 128]
    rem = M - sum(head) - sum(tail)
    if rem < 0:
        head, tail = [], [512, 256, 128]
        rem = M - sum(tail)
        if rem < 0:
            return [M - 256, 256] if M > 512 else [M]
    mids = []
    while rem > 0:
        c = min(1024, rem)
        if rem - c and rem - c < 256:
            c = rem - 256
        mids.append(c)
        rem -= c
    return head + mids + tail


def _chunks(total, step):
    out = []
    p = 0
    while p < total:
        c = min(step, total - p)
        out.append((p, c))
        p += c
    return out


def _build(M_task, mm_dtype=MM_DTYPE):
    assert mm_dtype == "bf16"
    blocks = _plan_blocks(M_task)
    # interleaved issue order: (b0,s0), (b0,s1), (b1,s0), ...
    sched = [(bi, s) for bi in range(len(blocks)) for s in range(S)]
    # flat x layout: per (block, slot) region [128, DC*xl], partition-major
    offs = {}
    off = 0
    starts = np.concatenate([[0], np.cumsum(blocks)]).astype(int)
    for bi, s in sched:
        xl = blocks[bi]
        offs[(bi, s)] = off
        off += 128 * DC * xl
    total_x = off

    nc = bacc.Bacc(None, target_bir_lowering=False)
    xL = nc.declare_dram_parameter("xL", [total_x], _BF16, isOutput=False)
    wcat = nc.declare_dram_parameter("wcat", [S, 128, WCOLS], _BF16, isOutput=False)
    bcat = nc.declare_dram_parameter("bcat", [128, S * 128], _F32, isOutput=False)
    outT = nc.declare_dram_parameter("outT", [S, C, M_task], _F32, isOutput=True)

    relu = mybir.ActivationFunctionType.Relu
    N_WARMUP = 10

    with TileContext(nc) as tc:
        with (
            tc.tile_pool(name="wpool", bufs=1) as wpool,
            tc.tile_pool(name="xpool", bufs=1) as xpool,
            tc.tile_pool(name="hpool", bufs=8) as hpool,
            tc.tile_pool(name="opool", bufs=1) as opool,
            tc.tile_pool(name="psum1", bufs=5, space="PSUM") as psum1,
            tc.tile_pool(name="psum2", bufs=2, space="PSUM") as psum2,
            tc.tile_pool(name="psumw", bufs=1, space="PSUM") as psumw,
        ):
            # weights + biases FIRST on the sync ring, ahead of the x
            # flood (~1.6us of stream): anywhere else they drain packet-
            # interleaved with x and land 10+us late, stalling slot-1.
            # Biases ship as one [128, 256] f32 DMA (values replicated to
            # 512B rows: 8B-row DMAs are descriptor-RMW-slow)
            w0t = wpool.tile([128, WCOLS], _BF16, tag="w0", name="wt0")
            nc.sync.dma_start(w0t, wcat[0])
            w1t = wpool.tile([128, WCOLS], _BF16, tag="w1", name="wt1")
            nc.sync.dma_start(w1t, wcat[1])
            bt = wpool.tile([128, S * 128], _F32, tag="bb", name="bt")
            nc.sync.dma_start(bt, bcat[:, :])
            wts = [
                (w0t, bt[:, 0:1], bt[0:C, 64:65]),
                (w1t, bt[:, 128:129], bt[0:C, 192:193]),
            ]

            # all x block DMAs up-front on the sync HWDGE ring, in consumption
            # order: the ring FIFO delivers blocks sequentially at line rate
            xts = {}
            for bi, s in sched:
                xl = blocks[bi]
                o = offs[(bi, s)]
                xt = xpool.tile(
                    [128, DC * xl], _BF16, tag=f"x{bi}_{s}", name=f"x{bi}_{s}"
                )
                nc.sync.dma_start(
                    xt, xL[o : o + 128 * DC * xl].rearrange("(p k) -> p k", p=128)
                )
                xts[(bi, s)] = xt

            # PE warmup on slot-0 weights (land ~9us; x flows from ~11us):
            # garbage matmuls into a scratch bank release the HAM clock gate
            # and bridge PE-busy until delivery is continuous
            wps = psumw.tile([128, MT], _F32, tag="wps")
            w0 = wts[0][0]
            for _ in range(N_WARMUP):
                nc.tensor.matmul(wps[:], w0[:, :128], w0[:, :MT], start=True, stop=True)

            # Software pipeline: wave w's L2 matmuls/bias/out are emitted
            # after wave w+1's L1 matmuls, so the in-order PE queue never
            # stalls on the relu ACT that produces ht.
            pending = None

            def flush_l2(nxt):
                nonlocal pending
                if pending is None:
                    pending = nxt
                    return
                pwt, phts, pps2, pwave, pot, pb2t, pouts = pending
                for j, (m0, mt) in enumerate(pwave):
                    nc.tensor.matmul(
                        pps2[32 * j : 32 * j + C, :mt],
                        pwt[:, DC * H : DC * H + C],
                        phts[j][:, :mt],
                        start=True,
                        stop=True,
                        tile_position=(0, 32 * j),
                    )
                for j, (m0, mt) in enumerate(pwave):
                    nc.vector.tensor_tensor(
                        pot[:, m0 : m0 + mt],
                        pps2[32 * j : 32 * j + C, :mt],
                        pb2t.to_broadcast([C, mt]),
                        mybir.AluOpType.add,
                    )
                for dma_args in pouts:
                    # sync ring: out descriptors queue behind the x stream
                    # and drain right after it; ot tiles are never recycled
                    # (distinct tags) so compute never waits on these
                    nc.sync.dma_start(*dma_args)
                pending = nxt

            WAVE = 2
            for bi, s in sched:
                xl = blocks[bi]
                x0 = starts[bi]
                wt, b1t, b2t = wts[s]
                xt = xts[(bi, s)]
                ot = opool.tile([C, xl], _F32, tag=f"o{bi}_{s}", name=f"o{bi}_{s}")
                subs = _chunks(xl, MT)
                for w0i in range(0, len(subs), WAVE):
                    wave = subs[w0i : w0i + WAVE]
                    ps1s = [
                        psum1.tile([H, MT], _F32, tag="ps1", name=f"ps1_{j}")
                        for j in range(len(wave))
                    ]
                    for dc in range(DC):
                        lhs = wt[:, dc * H : (dc + 1) * H]
                        for j, (m0, mt) in enumerate(wave):
                            nc.tensor.matmul(
                                ps1s[j][:, :mt],
                                lhs,
                                xt[:, dc * xl + m0 : dc * xl + m0 + mt],
                                start=(dc == 0),
                                stop=(dc == DC - 1),
                            )
                    last_wave = w0i + WAVE >= len(subs)
                    outs = (
                        [(outT[s, :, x0 : x0 + xl], ot[:])] if last_wave else []
                    )
                    hts = []
                    for j, (m0, mt) in enumerate(wave):
                        ht = hpool.tile([H, MT], _BF16, tag="h")
                        nc.scalar.activation(
                            ht[:, :mt], ps1s[j][:, :mt], relu, bias=b1t
                        )
                        hts.append(ht)
                    ps2 = psum2.tile([128, MT], _F32, tag="ps2")
                    flush_l2((wt, hts, ps2, wave, ot, b2t, outs))
            flush_l2(None)
    nc.compile()
    return nc


def _prepare(x, task_id, W1, b1, W2, b2, mm_dtype=MM_DTYPE):
    """Host-side routing: returns (in_maps, idx, counts, M_task)."""
    bf16 = _np_bf16()
    x = np.ascontiguousarray(np.asarray(x, dtype=np.float32))
    task_id = np.asarray(task_id).astype(np.int64)
    W1 = np.asarray(W1, dtype=np.float32)
    b1 = np.asarray(b1, dtype=np.float32)
    W2 = np.asarray(W2, dtype=np.float32)
    b2 = np.asarray(b2, dtype=np.float32)

    order = np.argsort(task_id, kind="stable")
    counts = np.bincount(task_id, minlength=T)
    starts_t = np.concatenate([[0], np.cumsum(counts)])
    M_task = max(128, int(-(-int(counts.max()) // 32) * 32))

    blocks = _plan_blocks(M_task)
    sched = [(bi, s) for bi in range(len(blocks)) for s in range(S)]
    bstarts = np.concatenate([[0], np.cumsum(blocks)]).astype(int)

    # idx[t] = sample rows for task t, padded with row 0 (discarded later)
    idx = np.zeros((T, M_task), dtype=np.int64)
    for t in range(T):
        idx[t, : counts[t]] = order[starts_t[t] : starts_t[t + 1]]

    in_maps = []
    for c in range(N_CORES):
        ts_c = [S * c + s for s in range(S)]
        # xT[s] = [DC, 128, M] (d-major within chunk on axis 1)
        xTs = []
        for s in range(S):
            xg = x[idx[ts_c[s]]].astype(bf16)  # [M, D]
            xTs.append(np.ascontiguousarray(xg.T).reshape(DC, 128, M_task))
        xL = np.empty(sum(128 * DC * b for b in blocks) * S, dtype=bf16)
        off = 0
        for bi, s in sched:
            xl = blocks[bi]
            x0 = bstarts[bi]
            # region [128, DC, xl] partition-major
            reg = xTs[s][:, :, x0 : x0 + xl].transpose(1, 0, 2)
            n = 128 * DC * xl
            xL[off : off + n] = reg.reshape(-1)
            off += n

        wcat = np.zeros((S, 128, WCOLS), dtype=bf16)
        bcat = np.zeros((128, S * 128), dtype=np.float32)
        for s in range(S):
            t = ts_c[s]
            # w1 [D,H] -> [128, DC*H] partition-major
            wcat[s, :, : DC * H] = (
                W1[t].reshape(DC, 128, H).transpose(1, 0, 2).reshape(128, DC * H)
            ).astype(bf16)
            wcat[s, :, DC * H : DC * H + C] = W2[t].astype(bf16)
            bcat[:, s * 128 : s * 128 + 64] = b1[t][:, None]
            bcat[:C, s * 128 + 64 : s * 128 + 128] = b2[t][:, None]

        in_maps.append({"xL": xL, "wcat": wcat, "bcat": bcat})
    return in_maps, idx, counts, M_task


def _unshard(results, idx, counts, b_total=B):
    out = np.empty((b_total, C), dtype=np.float32)
    for c in range(N_CORES):
        yT = np.asarray(results[c]["outT"])  # [S, C, M_task]
        y = yT.transpose(0, 2, 1)  # [S, M_task, C]
        for s in range(S):
            t = S * c + s
            cnt = counts[t]
            out[idx[t, :cnt]] = y[s, :cnt]
    return out


def kernel(x, task_id, W1, b1, W2, b2):
    in_maps, idx, counts, M_task = _prepare(x, task_id, W1, b1, W2, b2)
    nc = _build(M_task)
    try:
        res = run_bass_kernel_spmd(nc, in_maps, list(range(N_CORES)))
    except Exception:
        # transient NRT device hiccups (e.g. NRT_EXEC_UNIT_UNRECOVERABLE)
        # have been observed to succeed on retry
        res = run_bass_kernel_spmd(nc, in_maps, list(range(N_CORES)))
    return _unshard(res.results, idx, counts, b_total=np.asarray(task_id).shape[0])
